# revision 9
# baseline (speedup 1.0000x reference)
"""BiGCN (rumor detection) forward pass on 8 TRN2 NeuronCores.

Data-parallel over graphs: 16 graphs per core (contiguous 512-node slices).
Per graph, message passing is done as dense matmuls against the graph's
normalized adjacency matrix (built on-device in SBUF via gpsimd.local_scatter
from host-prepared padded index/value tables).  Feature transforms run at
full-rate fp32 (float32r moving operand, N=512); aggregation runs in fp16.
"""

import numpy as np

import concourse.bass as bass
import concourse.bacc as bacc
import concourse.mybir as mybir
import concourse.tile as tile
from concourse.bass_utils import run_bass_kernel_spmd

F32 = mybir.dt.float32
F32R = mybir.dt.float32r
F16 = mybir.dt.float16
I16 = mybir.dt.int16
I32 = mybir.dt.int32

N, B, E = 65536, 128, 524288
IN, HID, OUT, NCLS = 256, 128, 128, 4
MAX_HOP = 10
NPG = N // B            # 512 nodes per graph
NCORES = 8
GPC = B // NCORES       # 16 graphs per core
NPCORE = N // NCORES    # 8192 nodes per core
NCHUNK = NPG // 128     # 4 x 128-node chunks per graph

AF = mybir.ActivationFunctionType
ALU = mybir.AluOpType


# --------------------------------------------------------------------------
# Bass program (one core's SPMD program; all cores identical, shards differ)
# --------------------------------------------------------------------------

def build_program(W: int):
    nc = bacc.Bacc("TRN2", target_bir_lowering=False, debug=False,
                   num_devices=NCORES)

    def din(name, shape, dt=F32):
        return nc.dram_tensor(name, shape, dt, kind="ExternalInput").ap()

    def dout(name, shape, dt=F32):
        return nc.dram_tensor(name, shape, dt, kind="ExternalOutput").ap()

    x_d = din("x_sh", [NPCORE, IN])
    scat_d = din("scat", [GPC, 128, 2, NCHUNK, 2, W], I16)
    us_d = din("ustate", [GPC, 600])
    nhop_d = din("nhop", [1, GPC], I32)

    w1_d = [din("w1_td", [IN, HID], F16), din("w1_bu", [IN, HID], F16)]
    w2_d = [din("w2_td", [HID + IN, OUT], F16), din("w2_bu", [HID + IN, OUT], F16)]
    b1_d = [din("b1_td", [HID, 1]), din("b1_bu", [HID, 1])]
    b2_d = [din("b2_td", [OUT, 1]), din("b2_bu", [OUT, 1])]
    wu_d, ws_d, wd_d = din("wu", [HID, HID]), din("ws", [HID, HID]), din("wd", [HID, HID])
    wub_d, wsb_d, wdb_d = din("wub", [HID, 1]), din("wsb", [HID, 1]), din("wdb", [HID, 1])
    wu0_d, ws0_d, wd0_d = din("wu0", [1, HID]), din("ws0", [1, HID]), din("wd0", [1, HID])
    wu0b_d, ws0b_d, wd0b_d = din("wu0b", [HID, 1]), din("ws0b", [HID, 1]), din("wd0b", [HID, 1])
    wx_d = din("wx", [HID * 3, OUT + HID])
    wxb_d = din("wxb", [OUT + HID, 1])
    lu_d, ls_d, ld_d = din("lu", [HID, 1]), din("ls", [HID, 1]), din("ld", [HID, 1])
    lub_d, lsb_d, ldb_d = din("lub", [1, 1]), din("lsb", [1, 1]), din("ldb", [1, 1])
    fcw_d = din("fcw", [(OUT + HID) * 2, NCLS])
    fcb_d = din("fcb", [1, NCLS])
    rawab_d = din("rawab", [1, 2])
    id32_d = din("id32", [128, 128])
    id16_d = din("id16", [128, 128], F16)

    logp_d = dout("out_logp", [GPC, NCLS])
    uo_d = dout("out_uo", [1, MAX_HOP * GPC])
    so_d = dout("out_so", [1, MAX_HOP * GPC])
    do_d = dout("out_do", [1, MAX_HOP * GPC])

    from contextlib import ExitStack

    with tile.TileContext(nc) as tc, ExitStack() as ctx:
        const = ctx.enter_context(tc.tile_pool(name="const", bufs=1))
        xload = ctx.enter_context(tc.tile_pool(name="xload", bufs=2))
        sb = ctx.enter_context(tc.tile_pool(name="sb", bufs=2))
        sb3 = ctx.enter_context(tc.tile_pool(name="sb3", bufs=3))
        xtp = ctx.enter_context(tc.tile_pool(name="xtp", bufs=4))
        pers = ctx.enter_context(tc.tile_pool(name="pers", bufs=1))
        p_xt = ctx.enter_context(tc.tile_pool(name="p_xt", bufs=1, space="PSUM"))
        p_hT = ctx.enter_context(tc.tile_pool(name="p_hT", bufs=1, space="PSUM"))
        p_tp = ctx.enter_context(tc.tile_pool(name="p_tp", bufs=1, space="PSUM"))
        p_agg = ctx.enter_context(tc.tile_pool(name="p_agg", bufs=2, space="PSUM"))
        p_sm = ctx.enter_context(tc.tile_pool(name="p_sm", bufs=2, space="PSUM"))

        def cload(ap_dram, shape, dt=F32, tag=None):
            t = const.tile(shape, dt, tag=tag)
            nc.sync.dma_start(t[:], ap_dram)
            return t

        # ---- constants ----
        w1sb = [cload(w1_d[b].rearrange("(k p) m -> p k m", p=128),
                      [128, 2, HID], F16, tag=f"w1_{b}") for b in range(2)]
        w2sb = [cload(w2_d[b].rearrange("(k p) m -> p k m", p=128),
                      [128, 3, OUT], F16, tag=f"w2_{b}") for b in range(2)]
        b1sb = [cload(b1_d[b], [128, 1], tag=f"b1_{b}") for b in range(2)]
        b2sb = [cload(b2_d[b], [128, 1], tag=f"b2_{b}") for b in range(2)]
        wusb = cload(wu_d, [128, 128], tag="wu")
        wssb = cload(ws_d, [128, 128], tag="ws")
        wdsb = cload(wd_d, [128, 128], tag="wd")
        wubsb = cload(wub_d, [128, 1], tag="wub")
        wsbsb = cload(wsb_d, [128, 1], tag="wsb")
        wdbsb = cload(wdb_d, [128, 1], tag="wdb")
        wu0sb = cload(wu0_d, [1, 128], tag="wu0")
        wu0bsb = cload(wu0b_d, [128, 1], tag="wu0b")
        ws0bsb = cload(ws0b_d, [128, 1], tag="ws0b")
        wd0bsb = cload(wd0b_d, [128, 1], tag="wd0b")
        wxsb = cload(wx_d.rearrange("(k p) m -> p k m", p=128),
                     [128, 3, OUT + HID], tag="wx")
        wxbsb = cload(wxb_d.rearrange("(m p) o -> p m o", p=128),
                      [128, 2, 1], tag="wxb")
        lusb = cload(lu_d, [128, 1], tag="lu")
        lssb = cload(ls_d, [128, 1], tag="ls")
        ldsb = cload(ld_d, [128, 1], tag="ld")
        lubsb = cload(lub_d, [1, 1], tag="lub")
        lsbsb = cload(lsb_d, [1, 1], tag="lsb")
        ldbsb = cload(ldb_d, [1, 1], tag="ldb")
        fcwsb = cload(fcw_d.rearrange("(k p) c -> p k c", p=128),
                      [128, 4, NCLS], tag="fcw")
        fcbsb = cload(fcb_d, [1, NCLS], tag="fcb")
        rawabsb = cload(rawab_d, [1, 2], tag="rawab")
        id32sb = cload(id32_d, [128, 128], tag="id32")
        id16sb = cload(id16_d, [128, 128], F16, tag="id16")

        ones128 = const.tile([1, 128], F32, tag="ones128")
        nc.vector.memset(ones128[:], 1.0)

        # persistent accumulators
        fcrhs = [pers.tile([128, GPC], F32, tag=f"fcrhs{k}", name=f"fcrhs{k}")
                 for k in range(4)]
        UallT = pers.tile([128, MAX_HOP * GPC], F32, tag="UallT")
        SallT = pers.tile([128, MAX_HOP * GPC], F32, tag="SallT")
        DallT = pers.tile([128, MAX_HOP * GPC], F32, tag="DallT")

        # =================== hop chain (tiny, overlaps graph work) =========
        # alpha/beta scalars -> broadcast to [128,3] (alpha, beta, 1-a-b)
        sig = sb.tile([1, 2], F32, tag="sig")
        nc.scalar.activation(sig[:], rawabsb[:], AF.Sigmoid)
        vals3 = sb.tile([1, 3], F32, tag="vals3")
        nc.vector.tensor_copy(vals3[:, 0:2], sig[:])
        t11 = sb.tile([1, 1], F32, tag="t11")
        nc.vector.tensor_tensor(t11[:], sig[:, 0:1], sig[:, 1:2], ALU.add)
        nc.vector.tensor_scalar(vals3[:, 2:3], t11[:], -1.0, 1.0, ALU.mult, ALU.add)
        ps3 = p_sm.tile([128, 3], F32, tag="psm")
        nc.tensor.matmul(ps3[:], ones128[:], vals3[:])
        scal_bc = pers.tile([128, 3], F32, tag="scal_bc")
        nc.vector.tensor_copy(scal_bc[:], ps3[:])

        # u0 = user_state.sum((1,2)) as a row [1, GPC]
        us_sb = sb.tile([GPC, 600], F32, tag="us")
        nc.sync.dma_start(us_sb[:], us_d)
        us_sum = sb.tile([GPC, 1], F32, tag="us_sum")
        nc.vector.tensor_reduce(us_sum[:], us_sb[:], mybir.AxisListType.X, ALU.add)
        ps_u0 = p_sm.tile([1, GPC], F32, tag="psm")
        nc.tensor.matmul(ps_u0[:], us_sum[:], id32sb[0:GPC, 0:GPC])
        u0row = sb.tile([1, GPC], F32, tag="u0row")
        nc.scalar.copy(u0row[:], ps_u0[:])

        # U_ = u0 @ wu0 + wu0b   (HID-major: [128, GPC])
        ps_U0 = p_sm.tile([128, GPC], F32, tag="psm")
        nc.tensor.matmul(ps_U0[:], wu0sb[:], u0row[:])
        U_prev = pers.tile([128, GPC], F32, tag="U_prev")
        nc.vector.tensor_scalar(U_prev[:], ps_U0[:], wu0bsb[:], None, ALU.add)
        S_prev = pers.tile([128, GPC], F32, tag="S_prev")
        nc.vector.tensor_copy(S_prev[:], ws0bsb[:].broadcast_to([128, GPC]))
        D_prev = pers.tile([128, GPC], F32, tag="D_prev")
        nc.vector.tensor_copy(D_prev[:], wd0bsb[:].broadcast_to([128, GPC]))

        for k in range(MAX_HOP):
            sl = slice(k * GPC, (k + 1) * GPC)
            tmpU = sb.tile([128, GPC], F32, tag="tmpU")
            nc.vector.tensor_scalar(tmpU[:], U_prev[:], scal_bc[:, 2:3], None, ALU.mult)
            psu = p_sm.tile([128, GPC], F32, tag="psm")
            nc.tensor.matmul(psu[:], wusb[:], tmpU[:])
            nc.vector.tensor_scalar(UallT[:, sl], psu[:], wubsb[:], None, ALU.add)

            tmpS = sb.tile([128, GPC], F32, tag="tmpS")
            nc.vector.tensor_scalar(tmpS[:], UallT[:, sl], scal_bc[:, 0:1], None, ALU.mult)
            nc.vector.tensor_tensor(tmpS[:], tmpS[:], S_prev[:], ALU.add)
            pss = p_sm.tile([128, GPC], F32, tag="psm")
            nc.tensor.matmul(pss[:], wssb[:], tmpS[:])
            nc.vector.tensor_scalar(SallT[:, sl], pss[:], wsbsb[:], None, ALU.add)

            tmpD = sb.tile([128, GPC], F32, tag="tmpD")
            nc.vector.tensor_scalar(tmpD[:], UallT[:, sl], scal_bc[:, 1:2], None, ALU.mult)
            nc.vector.tensor_tensor(tmpD[:], tmpD[:], D_prev[:], ALU.add)
            psd = p_sm.tile([128, GPC], F32, tag="psm")
            nc.tensor.matmul(psd[:], wdsb[:], tmpD[:])
            nc.vector.tensor_scalar(DallT[:, sl], psd[:], wdbsb[:], None, ALU.add)

            U_prev, S_prev, D_prev = UallT[:, sl], SallT[:, sl], DallT[:, sl]

        # Uo/So/Do = U_all @ l*_w + l*_b  -> [1, MAX_HOP*GPC]
        for allT, lw, lb, od in ((UallT, lusb, lubsb, uo_d),
                                 (SallT, lssb, lsbsb, so_d),
                                 (DallT, ldsb, ldbsb, do_d)):
            pso = p_sm.tile([1, MAX_HOP * GPC], F32, tag="psm")
            nc.tensor.matmul(pso[:], lw[:], allT[:])
            osb = sb.tile([1, MAX_HOP * GPC], F32, tag="osb")
            nc.vector.tensor_scalar(osb[:], pso[:], lb[:], None, ALU.add)
            nc.sync.dma_start(od, osb[:])

        # hop one-hot selection -> U_m/S_m/D_m  [128, GPC]
        nhop_sb = sb.tile([1, GPC], I32, tag="nhop")
        nc.sync.dma_start(nhop_sb[:], nhop_d)
        nh1 = sb.tile([1, GPC], I32, tag="nh1")
        nc.vector.tensor_scalar(nh1[:], nhop_sb[:], 1, None, ALU.subtract)
        iota_t = sb.tile([1, MAX_HOP * GPC], I32, tag="iota")
        nc.gpsimd.iota(iota_t[:], [[1, MAX_HOP], [0, GPC]], channel_multiplier=0)
        oh_i = sb.tile([1, MAX_HOP * GPC], I32, tag="oh_i")
        nc.vector.tensor_tensor(
            oh_i[:].rearrange("p (h g) -> p h g", h=MAX_HOP),
            iota_t[:].rearrange("p (h g) -> p h g", h=MAX_HOP),
            nh1[:].unsqueeze(1).broadcast_to([1, MAX_HOP, GPC]),
            ALU.is_equal)
        oh_f = sb.tile([1, MAX_HOP * GPC], F32, tag="oh_f")
        nc.vector.tensor_copy(oh_f[:], oh_i[:])
        ps_oh = p_sm.tile([128, MAX_HOP * GPC], F32, tag="psm")
        nc.tensor.matmul(ps_oh[:], ones128[:], oh_f[:])
        ohB = sb.tile([128, MAX_HOP * GPC], F32, tag="ohB")
        nc.vector.tensor_copy(ohB[:], ps_oh[:])

        msel = []
        for allT, nm in ((UallT, "U"), (SallT, "S"), (DallT, "D")):
            msk = sb.tile([128, MAX_HOP * GPC], F32, tag="msk")
            nc.vector.tensor_tensor(msk[:], allT[:], ohB[:], ALU.mult)
            mt = pers.tile([128, GPC], F32, tag=f"m_{nm}")
            nc.vector.tensor_reduce(mt[:], msk[:].rearrange("p (h g) -> p g h", h=MAX_HOP),
                                    mybir.AxisListType.X, ALU.add)
            msel.append(mt)

        # xg = concat(U_m,S_m,D_m) @ wx + wxb  -> two [128, GPC] tiles
        xgT = []
        for m in range(2):
            psx = p_sm.tile([128, GPC], F32, tag="psm")
            for k in range(3):
                nc.tensor.matmul(psx[:], wxsb[:, k, 128 * m:128 * (m + 1)], msel[k][:],
                                 start=(k == 0), stop=(k == 2))
            xg_m = pers.tile([128, GPC], F32, tag=f"xg{m}")
            nc.vector.tensor_scalar(xg_m[:], psx[:], wxbsb[:, m, :], None, ALU.add)
            xgT.append(xg_m)

        # =================== per-graph GNN ================================
        for g in range(GPC):
            x_sb = xload.tile([128, NCHUNK, IN], F32, tag="x_sb")
            nc.sync.dma_start(
                x_sb[:], x_d[g * NPG:(g + 1) * NPG, :].rearrange("(c p) f -> p c f", p=128))
            scat_sb = xload.tile([128, 2, NCHUNK, 2, W], I16, tag="scat_sb")
            nc.sync.dma_start(scat_sb[:], scat_d[g])

            # cast x to fp16, then x^T (feature-major) via PE transpose
            x16 = xload.tile([128, NCHUNK, IN], F16, tag="x16")
            nc.scalar.copy(x16[:], x_sb[:])
            xT = []
            for i in range(2):
                pxt = p_xt.tile([128, NPG], F16, tag="pxt")
                for c in range(NCHUNK):
                    nc.tensor.transpose(pxt[:, 128 * c:128 * (c + 1)],
                                        x16[:, c, 128 * i:128 * (i + 1)], id16sb[:])
                xt_i = xtp.tile([128, NPG], F16, tag="xT")
                nc.vector.tensor_copy(xt_i[:], pxt[:])
                xT.append(xt_i)

            # root features (column 0), relu'd + broadcast (shared by branches)
            rhs2_sh = []
            for i in range(2):
                rroot = sb.tile([128, 1], F16, tag="rroot")
                nc.scalar.activation(rroot[:], xT[i][:, 0:1], AF.Relu)
                rb = sb.tile([128, NPG], F16, tag="rhs2_sh")
                nc.vector.tensor_copy(rb[:], rroot[:].broadcast_to([128, NPG]))
                rhs2_sh.append(rb)

            for b in range(2):
                # adjacency build (fp16) : 4 chunks of [128 src, 512 dst]
                ahat = sb3.tile([128, NCHUNK, NPG], F16, tag="ahat")
                for c in range(NCHUNK):
                    nc.gpsimd.local_scatter(
                        ahat[:, c, :],
                        scat_sb[:, b, c, 1, :].bitcast(F16),
                        scat_sb[:, b, c, 0, :],
                        channels=128, num_elems=NPG, num_idxs=W)

                # conv1 transform: h1T = W1^T @ xT  (fp16 full-rate)
                ph1 = p_hT.tile([128, NPG], F32, tag="ph")
                for k in range(2):
                    nc.tensor.matmul(ph1[:], w1sb[b][:, k, :], xT[k][:],
                                     start=(k == 0), stop=(k == 1))
                h1f16 = sb.tile([128, NPG], F16, tag="h1f16")
                nc.scalar.copy(h1f16[:], ph1[:])

                # transpose h1 -> node-major fp16 chunks
                ptp1 = p_tp.tile([128, NCHUNK, 128], F16, tag="ptp")
                for c in range(NCHUNK):
                    nc.tensor.transpose(ptp1[:, c, :], h1f16[:, 128 * c:128 * (c + 1)],
                                        id16sb[:])
                h1n = sb.tile([128, NCHUNK, 128], F16, tag="h1n")
                nc.vector.tensor_copy(h1n[:], ptp1[:])

                # aggregate 1
                pag1 = p_agg.tile([128, NPG], F32, tag="pagg")
                for c in range(NCHUNK):
                    nc.tensor.matmul(pag1[:], h1n[:, c, :], ahat[:, c, :],
                                     start=(c == 0), stop=(c == NCHUNK - 1))

                # conv2 inputs
                rhs2_0 = sb.tile([128, NPG], F16, tag="rhs2_0")
                nc.scalar.activation(rhs2_0[:], pag1[:], AF.Relu, bias=b1sb[b][:])
                x2root = sb.tile([128, 1], F32, tag="x2root")
                nc.vector.tensor_scalar(x2root[:], pag1[:, 0:1], b1sb[b][:], None, ALU.add)

                # conv2 transform (fp16)
                ph2 = p_hT.tile([128, NPG], F32, tag="ph")
                rhs2 = [rhs2_0, rhs2_sh[0], rhs2_sh[1]]
                for k in range(3):
                    nc.tensor.matmul(ph2[:], w2sb[b][:, k, :], rhs2[k][:],
                                     start=(k == 0), stop=(k == 2))
                h2f16 = sb.tile([128, NPG], F16, tag="h2f16")
                nc.scalar.copy(h2f16[:], ph2[:])

                ptp2 = p_tp.tile([128, NCHUNK, 128], F16, tag="ptp")
                for c in range(NCHUNK):
                    nc.tensor.transpose(ptp2[:, c, :], h2f16[:, 128 * c:128 * (c + 1)],
                                        id16sb[:])
                h2n = sb.tile([128, NCHUNK, 128], F16, tag="h2n")
                nc.vector.tensor_copy(h2n[:], ptp2[:])

                # aggregate 2
                pag2 = p_agg.tile([128, NPG], F32, tag="pagg")
                for c in range(NCHUNK):
                    nc.tensor.matmul(pag2[:], h2n[:, c, :], ahat[:, c, :],
                                     start=(c == 0), stop=(c == NCHUNK - 1))

                # out = relu(agg2 + b2); pooled mean via accum_out
                out2 = sb.tile([128, NPG], F32, tag="out2")
                pool_sum = sb.tile([128, 1], F32, tag="pool_sum")
                nc.scalar.activation(out2[:], pag2[:], AF.Relu, bias=b2sb[b][:],
                                     accum_out=pool_sum[:])

                # fc rhs columns: bu -> k0/k1, td -> k2/k3
                base = 2 if b == 0 else 0
                nc.vector.tensor_scalar(fcrhs[base][:, g:g + 1], pool_sum[:],
                                        1.0 / NPG, None, ALU.mult)
                nc.vector.tensor_copy(fcrhs[base + 1][:, g:g + 1], x2root[:])

        # =================== final FC + log_softmax =======================
        for k in range(4):
            nc.vector.tensor_tensor(fcrhs[k][:], fcrhs[k][:], xgT[k % 2][:], ALU.add)

        ps_log = p_sm.tile([GPC, NCLS], F32, tag="psm")
        for k in range(4):
            nc.tensor.matmul(ps_log[:], fcrhs[k][:], fcwsb[:, k, :],
                             start=(k == 0), stop=False)
        nc.tensor.matmul(ps_log[:], ones128[:, 0:GPC], fcbsb[:],
                         start=False, stop=True)

        negmax = sb.tile([GPC, 1], F32, tag="negmax")
        nc.vector.tensor_reduce(negmax[:], ps_log[:], mybir.AxisListType.X,
                                ALU.max, negate=True)
        exp_sb = sb.tile([GPC, NCLS], F32, tag="exp_sb")
        nc.scalar.activation(exp_sb[:], ps_log[:], AF.Exp, bias=negmax[:])
        ssum = sb.tile([GPC, 1], F32, tag="ssum")
        nc.vector.tensor_reduce(ssum[:], exp_sb[:], mybir.AxisListType.X, ALU.add)
        lse = sb.tile([GPC, 1], F32, tag="lse")
        nc.scalar.activation(lse[:], ssum[:], AF.Ln)
        mpl = sb.tile([GPC, 1], F32, tag="mpl")
        nc.vector.tensor_tensor(mpl[:], lse[:], negmax[:], ALU.subtract)
        lp_sb = sb.tile([GPC, NCLS], F32, tag="lp_sb")
        nc.vector.tensor_scalar(lp_sb[:], ps_log[:], mpl[:], None, ALU.subtract)
        nc.sync.dma_start(logp_d, lp_sb[:])

    nc.compile()
    return nc


# --------------------------------------------------------------------------
# Host-side preprocessing: shard + scatter-table build
# --------------------------------------------------------------------------

def _scatter_tables(src, dst):
    """Padded per-row (idx, val) tables for both branches.

    Returns idx [2, N, W] int16, val [2, N, W] fp16, W."""
    loop = np.arange(N, dtype=np.int64)
    s2 = np.concatenate([src, loop])
    d2 = np.concatenate([dst, loop])

    out = []
    W = 2
    for bs, bd in ((s2, d2), (d2, s2)):
        # row = message provider (bs), col = receiver local idx (bd % NPG)
        deg = np.bincount(bd, minlength=N).astype(np.float64)
        dinv = 1.0 / np.sqrt(deg)  # deg >= 1 thanks to self loops
        key = bs * NPG + (bd % NPG)
        uk, cnt = np.unique(key, return_counts=True)
        rows = uk // NPG
        cols = (uk % NPG).astype(np.int64)
        cols_g = (rows // NPG) * NPG + cols
        vals = cnt * dinv[rows] * dinv[cols_g]
        rc = np.bincount(rows, minlength=N)
        W = max(W, int(rc.max()))
        out.append((rows, cols, vals, rc))

    W = (W + 1) // 2 * 2
    idx_a = np.full((2, N, W), -1, np.int16)
    val_a = np.zeros((2, N, W), np.float16)
    for i, (rows, cols, vals, rc) in enumerate(out):
        offs = np.zeros(N + 1, np.int64)
        np.cumsum(rc, out=offs[1:])
        pos = np.arange(len(rows)) - offs[rows]
        idx_a[i, rows, pos] = cols.astype(np.int16)
        val_a[i, rows, pos] = vals.astype(np.float16)
    return idx_a, val_a, W


def _prep_inputs(x, user_state, params, edge_index, batch, num_hop):
    x = np.asarray(x, np.float32)
    user_state = np.asarray(user_state, np.float32)
    src = np.asarray(edge_index[0], np.int64)
    dst = np.asarray(edge_index[1], np.int64)
    num_hop = np.asarray(num_hop, np.int64)
    assert (src // NPG == dst // NPG).all(), "edges must be intra-graph"

    idx_a, val_a, W = _scatter_tables(src, dst)
    # node n -> (core, g, chunk, part):  n = ((core*GPC+g)*NCHUNK+c)*128+p
    idx_r = idx_a.reshape(2, NCORES, GPC, NCHUNK, 128, W)
    val_r = val_a.reshape(2, NCORES, GPC, NCHUNK, 128, W).view(np.int16)
    scat = np.empty((NCORES, GPC, 128, 2, NCHUNK, 2, W), np.int16)
    for b in range(2):
        # [core, g, c, p, w] -> [core, g, p, c, w]
        scat[:, :, :, b, :, 0, :] = idx_r[b].transpose(0, 1, 3, 2, 4)
        scat[:, :, :, b, :, 1, :] = val_r[b].transpose(0, 1, 3, 2, 4)

    p = {k: np.asarray(v, np.float32) for k, v in params.items()}
    shared = {
        "w1_td": p["td_w1"].astype(np.float16), "w1_bu": p["bu_w1"].astype(np.float16),
        "w2_td": p["td_w2"].astype(np.float16), "w2_bu": p["bu_w2"].astype(np.float16),
        "b1_td": p["td_b1"].reshape(HID, 1), "b1_bu": p["bu_b1"].reshape(HID, 1),
        "b2_td": p["td_b2"].reshape(OUT, 1), "b2_bu": p["bu_b2"].reshape(OUT, 1),
        "wu": p["wu_w"], "ws": p["ws_w"], "wd": p["wd_w"],
        "wub": p["wu_b"].reshape(HID, 1), "wsb": p["ws_b"].reshape(HID, 1),
        "wdb": p["wd_b"].reshape(HID, 1),
        "wu0": p["wu0_w"].reshape(1, HID), "ws0": p["ws0_w"].reshape(1, HID),
        "wd0": p["wd0_w"].reshape(1, HID),
        "wu0b": p["wu0_b"].reshape(HID, 1), "ws0b": p["ws0_b"].reshape(HID, 1),
        "wd0b": p["wd0_b"].reshape(HID, 1),
        "wx": p["wx_w"], "wxb": p["wx_b"].reshape(OUT + HID, 1),
        "lu": p["lu_w"], "ls": p["ls_w"], "ld": p["ld_w"],
        "lub": p["lu_b"].reshape(1, 1), "lsb": p["ls_b"].reshape(1, 1),
        "ldb": p["ld_b"].reshape(1, 1),
        "fcw": p["fc_w"], "fcb": p["fc_b"].reshape(1, NCLS),
        "rawab": np.array([[float(p["raw_alpha"][0]), float(p["raw_beta"][0])]],
                          np.float32),
        "id32": np.eye(128, dtype=np.float32),
        "id16": np.eye(128, dtype=np.float16),
    }

    in_maps = []
    for c in range(NCORES):
        m = dict(shared)
        m["x_sh"] = np.ascontiguousarray(x[c * NPCORE:(c + 1) * NPCORE])
        m["scat"] = np.ascontiguousarray(scat[c])
        m["ustate"] = np.ascontiguousarray(
            user_state[c * GPC:(c + 1) * GPC].reshape(GPC, 600))
        m["nhop"] = np.ascontiguousarray(
            num_hop[c * GPC:(c + 1) * GPC].reshape(1, GPC).astype(np.int32))
        in_maps.append(m)
    return in_maps, W


def _assemble(results):
    logp = np.concatenate([r["out_logp"] for r in results], 0)
    outs = []
    for key in ("out_uo", "out_so", "out_do"):
        per = [r[key].reshape(MAX_HOP, GPC).T.reshape(GPC, MAX_HOP, 1)
               for r in results]
        outs.append(np.concatenate(per, 0))
    return (logp, outs[0], outs[1], outs[2])


_CACHE = {}


def kernel(x, user_state, params, edge_index, batch, num_hop, _want_stats=False):
    in_maps, W = _prep_inputs(x, user_state, params, edge_index, batch, num_hop)
    if W not in _CACHE:
        _CACHE[W] = build_program(W)
    nc = _CACHE[W]
    res = run_bass_kernel_spmd(nc, in_maps, core_ids=list(range(NCORES)))
    out = _assemble(res.results)
    if _want_stats:
        return out, res
    return out


# revision 10
# speedup vs baseline: 1.0912x; 1.0912x over previous
"""BiGCN (rumor detection) forward pass on 8 TRN2 NeuronCores.

Data-parallel over graphs: 16 graphs per core (contiguous 512-node slices).
Per graph, GCN message passing runs as dense matmuls against the graph's
normalized adjacency (built on-device in SBUF via gpsimd.local_scatter from
host-prepared padded index/value tables).  All feature math runs on device
in fp16 (fp32 accumulation); host work is limited to sharding and index
preprocessing (edge sort/merge, degree counts, layout packing).

Key structure choices:
  - x is shipped feature-major ([256, nodes], fp16) so conv1 consumes it
    directly as the stationary operand -> no on-device transposes at all.
  - conv outputs are node-major (x^T/h^T blocks stationary, W moving), so
    aggregation lhsT chunks come straight from the conv PSUM.
  - conv2's root-broadcast rows contribute a rank-1 term: c2 = W2[128:]^T @
    relu(root) is computed once per graph/branch ([128,128]-cheap) and added
    during the PSUM evacuation, so conv2 contracts only 128 of 384 features.
  - all small weights ride in ONE packed DMA blob (HWDGE overhead is ~625ns
    per dma_start; 35 separate loads would burn ~22us of startup).
"""

import numpy as np

import concourse.bass as bass
import concourse.bacc as bacc
import concourse.mybir as mybir
import concourse.tile as tile
from concourse.bass_utils import run_bass_kernel_spmd

F32 = mybir.dt.float32
F16 = mybir.dt.float16
I16 = mybir.dt.int16
I32 = mybir.dt.int32

N, B, E = 65536, 128, 524288
IN, HID, OUT, NCLS = 256, 128, 128, 4
MAX_HOP = 10
NPG = N // B            # 512 nodes per graph
NCORES = 8
GPC = B // NCORES       # 16 graphs per core
NPCORE = N // NCORES    # 8192 nodes per core
NCHUNK = NPG // 128     # 4 x 128-node chunks per graph

AF = mybir.ActivationFunctionType
ALU = mybir.AluOpType


# --------------------------------------------------------------------------
# Packed constant-blob layout (shared by host packing and device slicing)
# --------------------------------------------------------------------------

def _f32_layout():
    """name -> (col0, rows, ncols); column-packed [128, CF32] f32 blob."""
    lay = {}
    c = 0

    def add(name, rows, ncols):
        nonlocal c
        lay[name] = (c, rows, ncols)
        c += ncols

    add("wu", 128, 128)
    add("ws", 128, 128)
    add("wd", 128, 128)
    add("wub", 128, 1)
    add("wsb", 128, 1)
    add("wdb", 128, 1)
    add("wu0b", 128, 1)
    add("ws0b", 128, 1)
    add("wd0b", 128, 1)
    add("b1_td", 128, 1)
    add("b1_bu", 128, 1)
    add("b2_td", 128, 1)
    add("b2_bu", 128, 1)
    add("wx", 128, 3 * (OUT + HID))      # [128, 3, 256] k-major
    add("wxb2", 128, 2)                  # [128, 2] (two 128-halves of wx_b)
    add("lu", 128, 1)
    add("ls", 128, 1)
    add("ld", 128, 1)
    add("fcw", 128, 4 * NCLS)            # [128, 4, 4] k-major
    add("id16x16", 16, 16)
    add("wu0", 1, 128)
    add("lub", 1, 1)
    add("lsb", 1, 1)
    add("ldb", 1, 1)
    add("fcb", 1, NCLS)
    add("rawab", 1, 2)
    add("nhop", 1, GPC)                  # int32 bits
    add("us", GPC, 600)
    return lay, c


def _f16_layout():
    lay = {}
    c = 0

    def add(name, ncols):
        nonlocal c
        lay[name] = (c, ncols)
        c += ncols

    add("w1_td", 2 * HID)                # [128, 2, 128] k-major
    add("w1_bu", 2 * HID)
    add("w2_td", 3 * OUT)                # [128, 3, 128] k-major
    add("w2_bu", 3 * OUT)
    return lay, c


F32LAY, CF32 = _f32_layout()
F16LAY, CF16 = _f16_layout()


# --------------------------------------------------------------------------
# Bass program (one core's SPMD program; all cores identical, shards differ)
# --------------------------------------------------------------------------

def build_program(W: int):
    nc = bacc.Bacc("TRN2", target_bir_lowering=False, debug=False,
                   num_devices=NCORES)

    xt_d = nc.dram_tensor("xt_sh", [IN, NPCORE], F16, kind="ExternalInput").ap()
    scat_d = nc.dram_tensor("scat", [GPC, 128, 2, NCHUNK, 2, W], I16,
                            kind="ExternalInput").ap()
    cb32_d = nc.dram_tensor("cb32", [128, CF32], F32, kind="ExternalInput").ap()
    cb16_d = nc.dram_tensor("cb16", [128, CF16], F16, kind="ExternalInput").ap()

    logp_d = nc.dram_tensor("out_logp", [GPC, NCLS], F32, kind="ExternalOutput").ap()
    usd_d = [nc.dram_tensor(f"out_{k}", [1, MAX_HOP * GPC], F32,
                            kind="ExternalOutput").ap() for k in ("uo", "so", "do")]

    from contextlib import ExitStack

    with tile.TileContext(nc) as tc, ExitStack() as ctx:
        const = ctx.enter_context(tc.tile_pool(name="const", bufs=1))
        xload = ctx.enter_context(tc.tile_pool(name="xload", bufs=3))
        sb = ctx.enter_context(tc.tile_pool(name="sb", bufs=2))
        sb3 = ctx.enter_context(tc.tile_pool(name="sb3", bufs=3))
        pers = ctx.enter_context(tc.tile_pool(name="pers", bufs=1))
        p_h = ctx.enter_context(tc.tile_pool(name="p_h", bufs=3, space="PSUM"))
        p_agg = ctx.enter_context(tc.tile_pool(name="p_agg", bufs=2, space="PSUM"))
        p_sm = ctx.enter_context(tc.tile_pool(name="p_sm", bufs=3, space="PSUM"))

        cb32 = const.tile([128, CF32], F32, tag="cb32")
        nc.sync.dma_start(cb32[:], cb32_d)
        cb16 = const.tile([128, CF16], F16, tag="cb16")
        nc.sync.dma_start(cb16[:], cb16_d)

        def c32(name, reshape=None):
            c0, rows, ncols = F32LAY[name]
            ap = cb32[0:rows, c0:c0 + ncols]
            if reshape:
                ap = ap.rearrange(reshape[0], **reshape[1])
            return ap

        def c16(name, k):
            c0, ncols = F16LAY[name]
            return cb16[:, c0:c0 + ncols].rearrange("p (k m) -> p k m", k=k)

        w1sb = [c16("w1_td", 2), c16("w1_bu", 2)]
        w2sb = [c16("w2_td", 3), c16("w2_bu", 3)]
        b1sb = [c32("b1_td"), c32("b1_bu")]
        b2sb = [c32("b2_td"), c32("b2_bu")]
        wxsb = c32("wx", ("p (k m) -> p k m", {"k": 3}))
        fcwsb = c32("fcw", ("p (k m) -> p k m", {"k": 4}))

        ones128 = const.tile([1, 128], F32, tag="ones128")
        nc.vector.memset(ones128[:], 1.0)

        # persistent accumulators
        fcrhs = [pers.tile([128, GPC], F32, tag=f"fcrhs{k}", name=f"fcrhs{k}")
                 for k in range(4)]
        UallT = pers.tile([128, MAX_HOP * GPC], F32, tag="UallT")
        SallT = pers.tile([128, MAX_HOP * GPC], F32, tag="SallT")
        DallT = pers.tile([128, MAX_HOP * GPC], F32, tag="DallT")

        # =================== hop chain (tiny, overlaps graph work) =========
        sig = sb.tile([1, 2], F32, tag="sig")
        nc.scalar.activation(sig[:], c32("rawab"), AF.Sigmoid)
        vals3 = sb.tile([1, 3], F32, tag="vals3")
        nc.vector.tensor_copy(vals3[:, 0:2], sig[:])
        t11 = sb.tile([1, 1], F32, tag="t11")
        nc.vector.tensor_tensor(t11[:], sig[:, 0:1], sig[:, 1:2], ALU.add)
        nc.vector.tensor_scalar(vals3[:, 2:3], t11[:], -1.0, 1.0, ALU.mult, ALU.add)
        ps3 = p_sm.tile([128, 3], F32, tag="psm")
        nc.tensor.matmul(ps3[:], ones128[:], vals3[:])
        scal_bc = pers.tile([128, 3], F32, tag="scal_bc")
        nc.vector.tensor_copy(scal_bc[:], ps3[:])

        us_sum = sb.tile([GPC, 1], F32, tag="us_sum")
        nc.vector.tensor_reduce(us_sum[:], c32("us"), mybir.AxisListType.X, ALU.add)
        ps_u0 = p_sm.tile([1, GPC], F32, tag="psm")
        nc.tensor.matmul(ps_u0[:], us_sum[:], c32("id16x16"))
        u0row = sb.tile([1, GPC], F32, tag="u0row")
        nc.scalar.copy(u0row[:], ps_u0[:])

        ps_U0 = p_sm.tile([128, GPC], F32, tag="psm")
        nc.tensor.matmul(ps_U0[:], c32("wu0"), u0row[:])
        U_prev = pers.tile([128, GPC], F32, tag="U_prev")
        nc.vector.tensor_scalar(U_prev[:], ps_U0[:], c32("wu0b"), None, ALU.add)
        S_prev = pers.tile([128, GPC], F32, tag="S_prev")
        nc.vector.tensor_copy(S_prev[:], c32("ws0b").broadcast_to([128, GPC]))
        D_prev = pers.tile([128, GPC], F32, tag="D_prev")
        nc.vector.tensor_copy(D_prev[:], c32("wd0b").broadcast_to([128, GPC]))

        for k in range(MAX_HOP):
            sl = slice(k * GPC, (k + 1) * GPC)
            tmpU = sb.tile([128, GPC], F32, tag="tmpU")
            nc.vector.tensor_scalar(tmpU[:], U_prev[:], scal_bc[:, 2:3], None, ALU.mult)
            psu = p_sm.tile([128, GPC], F32, tag="psm")
            nc.tensor.matmul(psu[:], c32("wu"), tmpU[:])
            nc.vector.tensor_scalar(UallT[:, sl], psu[:], c32("wub"), None, ALU.add)

            tmpS = sb.tile([128, GPC], F32, tag="tmpS")
            nc.vector.tensor_scalar(tmpS[:], UallT[:, sl], scal_bc[:, 0:1], None, ALU.mult)
            nc.vector.tensor_tensor(tmpS[:], tmpS[:], S_prev[:], ALU.add)
            pss = p_sm.tile([128, GPC], F32, tag="psm")
            nc.tensor.matmul(pss[:], c32("ws"), tmpS[:])
            nc.vector.tensor_scalar(SallT[:, sl], pss[:], c32("wsb"), None, ALU.add)

            tmpD = sb.tile([128, GPC], F32, tag="tmpD")
            nc.vector.tensor_scalar(tmpD[:], UallT[:, sl], scal_bc[:, 1:2], None, ALU.mult)
            nc.vector.tensor_tensor(tmpD[:], tmpD[:], D_prev[:], ALU.add)
            psd = p_sm.tile([128, GPC], F32, tag="psm")
            nc.tensor.matmul(psd[:], c32("wd"), tmpD[:])
            nc.vector.tensor_scalar(DallT[:, sl], psd[:], c32("wdb"), None, ALU.add)

            U_prev, S_prev, D_prev = UallT[:, sl], SallT[:, sl], DallT[:, sl]

        for allT, lw, lb, od in ((UallT, "lu", "lub", usd_d[0]),
                                 (SallT, "ls", "lsb", usd_d[1]),
                                 (DallT, "ld", "ldb", usd_d[2])):
            pso = p_sm.tile([1, MAX_HOP * GPC], F32, tag="psm")
            nc.tensor.matmul(pso[:], c32(lw), allT[:])
            osb = sb.tile([1, MAX_HOP * GPC], F32, tag="osb")
            nc.vector.tensor_scalar(osb[:], pso[:], c32(lb), None, ALU.add)
            nc.sync.dma_start(od, osb[:])

        # hop one-hot selection -> U_m/S_m/D_m  [128, GPC]
        nh1 = sb.tile([1, GPC], I32, tag="nh1")
        nc.vector.tensor_scalar(nh1[:], c32("nhop").bitcast(I32), 1, None, ALU.subtract)
        iota_t = sb.tile([1, MAX_HOP * GPC], I32, tag="iota")
        nc.gpsimd.iota(iota_t[:], [[1, MAX_HOP], [0, GPC]], channel_multiplier=0)
        oh_i = sb.tile([1, MAX_HOP * GPC], I32, tag="oh_i")
        nc.vector.tensor_tensor(
            oh_i[:].rearrange("p (h g) -> p h g", h=MAX_HOP),
            iota_t[:].rearrange("p (h g) -> p h g", h=MAX_HOP),
            nh1[:].unsqueeze(1).broadcast_to([1, MAX_HOP, GPC]),
            ALU.is_equal)
        oh_f = sb.tile([1, MAX_HOP * GPC], F32, tag="oh_f")
        nc.vector.tensor_copy(oh_f[:], oh_i[:])
        ps_oh = p_sm.tile([128, MAX_HOP * GPC], F32, tag="psm")
        nc.tensor.matmul(ps_oh[:], ones128[:], oh_f[:])
        ohB = sb.tile([128, MAX_HOP * GPC], F32, tag="ohB")
        nc.vector.tensor_copy(ohB[:], ps_oh[:])

        msel = []
        for allT, nm in ((UallT, "U"), (SallT, "S"), (DallT, "D")):
            msk = sb.tile([128, MAX_HOP * GPC], F32, tag="msk")
            nc.vector.tensor_tensor(msk[:], allT[:], ohB[:], ALU.mult)
            mt = pers.tile([128, GPC], F32, tag=f"m_{nm}")
            nc.vector.tensor_reduce(mt[:], msk[:].rearrange("p (h g) -> p g h", h=MAX_HOP),
                                    mybir.AxisListType.X, ALU.add)
            msel.append(mt)

        xgT = []
        for m in range(2):
            psx = p_sm.tile([128, GPC], F32, tag="psm")
            for k in range(3):
                nc.tensor.matmul(psx[:], wxsb[:, k, 128 * m:128 * (m + 1)], msel[k][:],
                                 start=(k == 0), stop=(k == 2))
            xg_m = pers.tile([128, GPC], F32, tag=f"xg{m}", name=f"xg{m}")
            nc.vector.tensor_scalar(xg_m[:], psx[:], cb32[0:128, F32LAY["wxb2"][0] + m:
                                                          F32LAY["wxb2"][0] + m + 1],
                                    None, ALU.add)
            xgT.append(xg_m)

        # =================== per-graph GNN ================================
        for g in range(GPC):
            # feature-major x chunks, fp16, straight from HBM
            xT = []
            for i in range(2):
                xt_i = xload.tile([128, NPG], F16, tag="xT", name=f"xT{i}")
                nc.sync.dma_start(
                    xt_i[:], xt_d[128 * i:128 * (i + 1), g * NPG:(g + 1) * NPG])
                xT.append(xt_i)
            scat_sb = xload.tile([128, 2, NCHUNK, 2, W], I16, tag="scat_sb")
            nc.sync.dma_start(scat_sb[:], scat_d[g])

            # root feature column (node 0), relu'd, + [128,128] broadcasts
            rbc = []
            for i in range(2):
                rroot = sb.tile([128, 1], F16, tag="rroot")
                nc.scalar.activation(rroot[:], xT[i][:, 0:1], AF.Relu)
                rb = sb.tile([128, 128], F16, tag="rbc")
                nc.vector.tensor_copy(rb[:], rroot[:].broadcast_to([128, 128]))
                rbc.append(rb)

            for b in range(2):
                # adjacency build (fp16): 4 chunks of [128 src, 512 dst]
                ahat = sb3.tile([128, NCHUNK, NPG], F16, tag="ahat")
                for c in range(NCHUNK):
                    nc.gpsimd.local_scatter(
                        ahat[:, c, :],
                        scat_sb[:, b, c, 1, :].bitcast(F16),
                        scat_sb[:, b, c, 0, :],
                        channels=128, num_elems=NPG, num_idxs=W)

                # conv1, node-major: x^T blocks stationary, W1 moving
                ph1 = p_h.tile([128, NCHUNK, HID], F32, tag="ph")
                for j in range(NCHUNK):
                    for k in range(2):
                        nc.tensor.matmul(ph1[:, j, :],
                                         xT[k][:, 128 * j:128 * (j + 1)],
                                         w1sb[b][:, k, :],
                                         start=(k == 0), stop=(k == 1))
                h1n = sb.tile([128, NCHUNK, HID], F16, tag="h1n")
                nc.scalar.copy(h1n[:], ph1[:])

                # aggregate 1
                pag1 = p_agg.tile([128, NPG], F32, tag="pagg")
                for c in range(NCHUNK):
                    nc.tensor.matmul(pag1[:], h1n[:, c, :], ahat[:, c, :],
                                     start=(c == 0), stop=(c == NCHUNK - 1))

                # conv2 inputs: relu(agg1 + b1) (feat rows 0-127) + rank-1 root term
                rhs2_0 = sb.tile([128, NPG], F16, tag="rhs2_0")
                nc.scalar.activation(rhs2_0[:], pag1[:], AF.Relu, bias=b1sb[b])
                x2root = sb.tile([128, 1], F32, tag="x2root")
                nc.vector.tensor_scalar(x2root[:], pag1[:, 0:1], b1sb[b], None, ALU.add)

                # c2 = W2[128:384]^T @ relu(root): all rows equal c2
                pc2 = p_sm.tile([128, HID], F32, tag="psm")
                for k in (1, 2):
                    nc.tensor.matmul(pc2[:], rbc[k - 1][:], w2sb[b][:, k, :],
                                     start=(k == 1), stop=(k == 2))
                c2sb = sb.tile([128, HID], F32, tag="c2sb")
                nc.vector.tensor_copy(c2sb[:], pc2[:])

                # conv2, node-major, single 128-feature contraction
                ph2 = p_h.tile([128, NCHUNK, OUT], F32, tag="ph")
                for j in range(NCHUNK):
                    nc.tensor.matmul(ph2[:, j, :],
                                     rhs2_0[:, 128 * j:128 * (j + 1)],
                                     w2sb[b][:, 0, :])
                h2n = sb.tile([128, NCHUNK, OUT], F16, tag="h2n")
                nc.vector.tensor_tensor(h2n[:], ph2[:],
                                        c2sb[:].unsqueeze(1).broadcast_to(
                                            [128, NCHUNK, OUT]),
                                        ALU.add)

                # aggregate 2
                pag2 = p_agg.tile([128, NPG], F32, tag="pagg")
                for c in range(NCHUNK):
                    nc.tensor.matmul(pag2[:], h2n[:, c, :], ahat[:, c, :],
                                     start=(c == 0), stop=(c == NCHUNK - 1))

                # out = relu(agg2 + b2); pooled mean via accum_out
                out2 = sb.tile([128, NPG], F16, tag="out2")
                pool_sum = sb.tile([128, 1], F32, tag="pool_sum")
                nc.scalar.activation(out2[:], pag2[:], AF.Relu, bias=b2sb[b],
                                     accum_out=pool_sum[:])

                base = 2 if b == 0 else 0
                nc.vector.tensor_scalar(fcrhs[base][:, g:g + 1], pool_sum[:],
                                        1.0 / NPG, None, ALU.mult)
                nc.vector.tensor_copy(fcrhs[base + 1][:, g:g + 1], x2root[:])

        # =================== final FC + log_softmax =======================
        for k in range(4):
            nc.vector.tensor_tensor(fcrhs[k][:], fcrhs[k][:], xgT[k % 2][:], ALU.add)

        ps_log = p_sm.tile([GPC, NCLS], F32, tag="psm")
        for k in range(4):
            nc.tensor.matmul(ps_log[:], fcrhs[k][:], fcwsb[:, k, :],
                             start=(k == 0), stop=False)
        nc.tensor.matmul(ps_log[:], ones128[:, 0:GPC], c32("fcb"),
                         start=False, stop=True)

        negmax = sb.tile([GPC, 1], F32, tag="negmax")
        nc.vector.tensor_reduce(negmax[:], ps_log[:], mybir.AxisListType.X,
                                ALU.max, negate=True)
        exp_sb = sb.tile([GPC, NCLS], F32, tag="exp_sb")
        nc.scalar.activation(exp_sb[:], ps_log[:], AF.Exp, bias=negmax[:])
        ssum = sb.tile([GPC, 1], F32, tag="ssum")
        nc.vector.tensor_reduce(ssum[:], exp_sb[:], mybir.AxisListType.X, ALU.add)
        lse = sb.tile([GPC, 1], F32, tag="lse")
        nc.scalar.activation(lse[:], ssum[:], AF.Ln)
        mpl = sb.tile([GPC, 1], F32, tag="mpl")
        nc.vector.tensor_tensor(mpl[:], lse[:], negmax[:], ALU.subtract)
        lp_sb = sb.tile([GPC, NCLS], F32, tag="lp_sb")
        nc.vector.tensor_scalar(lp_sb[:], ps_log[:], mpl[:], None, ALU.subtract)
        nc.sync.dma_start(logp_d, lp_sb[:])

    nc.compile()
    return nc


# --------------------------------------------------------------------------
# Host-side preprocessing: shard + scatter-table build + blob packing
# --------------------------------------------------------------------------

def _scatter_tables(src, dst):
    """Padded per-row (idx, val) tables for both branches."""
    loop = np.arange(N, dtype=np.int64)
    s2 = np.concatenate([src, loop])
    d2 = np.concatenate([dst, loop])

    out = []
    W = 2
    for bs, bd in ((s2, d2), (d2, s2)):
        # row = message provider (bs), col = receiver local idx (bd % NPG)
        deg = np.bincount(bd, minlength=N).astype(np.float64)
        dinv = 1.0 / np.sqrt(deg)  # deg >= 1 thanks to self loops
        key = bs * NPG + (bd % NPG)
        uk, cnt = np.unique(key, return_counts=True)
        rows = uk // NPG
        cols = (uk % NPG).astype(np.int64)
        cols_g = (rows // NPG) * NPG + cols
        vals = cnt * dinv[rows] * dinv[cols_g]
        rc = np.bincount(rows, minlength=N)
        W = max(W, int(rc.max()))
        out.append((rows, cols, vals, rc))

    W = (W + 1) // 2 * 2
    idx_a = np.full((2, N, W), -1, np.int16)
    val_a = np.zeros((2, N, W), np.float16)
    for i, (rows, cols, vals, rc) in enumerate(out):
        offs = np.zeros(N + 1, np.int64)
        np.cumsum(rc, out=offs[1:])
        pos = np.arange(len(rows)) - offs[rows]
        idx_a[i, rows, pos] = cols.astype(np.int16)
        val_a[i, rows, pos] = vals.astype(np.float16)
    return idx_a, val_a, W


def _pack_blobs(params, user_state, num_hop):
    p = {k: np.asarray(v, np.float32) for k, v in params.items()}
    b32 = np.zeros((128, CF32), np.float32)

    def put(name, arr):
        c0, rows, ncols = F32LAY[name]
        arr = np.asarray(arr, np.float32).reshape(rows, ncols)
        b32[0:rows, c0:c0 + ncols] = arr

    put("wu", p["wu_w"]); put("ws", p["ws_w"]); put("wd", p["wd_w"])
    put("wub", p["wu_b"].reshape(128, 1)); put("wsb", p["ws_b"].reshape(128, 1))
    put("wdb", p["wd_b"].reshape(128, 1))
    put("wu0b", p["wu0_b"].reshape(128, 1)); put("ws0b", p["ws0_b"].reshape(128, 1))
    put("wd0b", p["wd0_b"].reshape(128, 1))
    put("b1_td", p["td_b1"].reshape(128, 1)); put("b1_bu", p["bu_b1"].reshape(128, 1))
    put("b2_td", p["td_b2"].reshape(128, 1)); put("b2_bu", p["bu_b2"].reshape(128, 1))
    # wx [384, 256] -> [128, 3, 256] k-major
    put("wx", p["wx_w"].reshape(3, 128, OUT + HID).transpose(1, 0, 2).reshape(128, -1))
    put("wxb2", p["wx_b"].reshape(2, 128).T)
    put("lu", p["lu_w"]); put("ls", p["ls_w"]); put("ld", p["ld_w"])
    put("fcw", p["fc_w"].reshape(4, 128, NCLS).transpose(1, 0, 2).reshape(128, -1))
    put("id16x16", np.eye(GPC, dtype=np.float32))
    put("wu0", p["wu0_w"].reshape(1, 128))
    put("lub", p["lu_b"].reshape(1, 1)); put("lsb", p["ls_b"].reshape(1, 1))
    put("ldb", p["ld_b"].reshape(1, 1))
    put("fcb", p["fc_b"].reshape(1, NCLS))
    put("rawab", np.array([[p["raw_alpha"][0], p["raw_beta"][0]]], np.float32))

    b16 = np.zeros((128, CF16), np.float16)

    def put16(name, arr, k):
        c0, ncols = F16LAY[name]
        a = np.asarray(arr, np.float32).reshape(k, 128, -1).transpose(1, 0, 2)
        b16[:, c0:c0 + ncols] = a.reshape(128, ncols).astype(np.float16)

    put16("w1_td", p["td_w1"], 2); put16("w1_bu", p["bu_w1"], 2)
    put16("w2_td", p["td_w2"], 3); put16("w2_bu", p["bu_w2"], 3)

    # per-core f32 blobs differ in nhop/us
    blobs32 = []
    c0, _, _ = F32LAY["nhop"]
    cu, _, _ = F32LAY["us"]
    for c in range(NCORES):
        bb = b32.copy()
        nh = np.asarray(num_hop[c * GPC:(c + 1) * GPC], np.int32)
        bb[0, c0:c0 + GPC] = nh.view(np.float32)
        bb[0:GPC, cu:cu + 600] = np.asarray(
            user_state[c * GPC:(c + 1) * GPC], np.float32).reshape(GPC, 600)
        blobs32.append(bb)
    return blobs32, b16


def _prep_inputs(x, user_state, params, edge_index, batch, num_hop):
    x = np.asarray(x, np.float32)
    user_state = np.asarray(user_state, np.float32)
    src = np.asarray(edge_index[0], np.int64)
    dst = np.asarray(edge_index[1], np.int64)
    num_hop = np.asarray(num_hop, np.int64)
    assert (src // NPG == dst // NPG).all(), "edges must be intra-graph"

    idx_a, val_a, W = _scatter_tables(src, dst)
    idx_r = idx_a.reshape(2, NCORES, GPC, NCHUNK, 128, W)
    val_r = val_a.reshape(2, NCORES, GPC, NCHUNK, 128, W).view(np.int16)
    scat = np.empty((NCORES, GPC, 128, 2, NCHUNK, 2, W), np.int16)
    for b in range(2):
        # [core, g, c, p, w] -> [core, g, p, c, w]
        scat[:, :, :, b, :, 0, :] = idx_r[b].transpose(0, 1, 3, 2, 4)
        scat[:, :, :, b, :, 1, :] = val_r[b].transpose(0, 1, 3, 2, 4)

    xt16 = np.ascontiguousarray(x.T.astype(np.float16))  # [256, N]
    blobs32, b16 = _pack_blobs(params, user_state, num_hop)

    in_maps = []
    for c in range(NCORES):
        in_maps.append({
            "xt_sh": np.ascontiguousarray(xt16[:, c * NPCORE:(c + 1) * NPCORE]),
            "scat": np.ascontiguousarray(scat[c]),
            "cb32": blobs32[c],
            "cb16": b16,
        })
    return in_maps, W


def _assemble(results):
    logp = np.concatenate([r["out_logp"] for r in results], 0)
    outs = []
    for key in ("out_uo", "out_so", "out_do"):
        per = [r[key].reshape(MAX_HOP, GPC).T.reshape(GPC, MAX_HOP, 1)
               for r in results]
        outs.append(np.concatenate(per, 0))
    return (logp, outs[0], outs[1], outs[2])


_CACHE = {}


def kernel(x, user_state, params, edge_index, batch, num_hop, _want_stats=False):
    in_maps, W = _prep_inputs(x, user_state, params, edge_index, batch, num_hop)
    if W not in _CACHE:
        _CACHE[W] = build_program(W)
    nc = _CACHE[W]
    res = run_bass_kernel_spmd(nc, in_maps, core_ids=list(range(NCORES)))
    out = _assemble(res.results)
    if _want_stats:
        return out, res
    return out


# revision 18
# speedup vs baseline: 1.2764x; 1.1697x over previous
"""BiGCN (rumor detection) forward pass on 8 TRN2 NeuronCores.

Data-parallel over graphs: 16 graphs per core (contiguous 512-node slices).
Per graph, GCN message passing runs as dense matmuls against the graph's
normalized adjacency (built on-device in SBUF via gpsimd.local_scatter from
host-prepared padded index/value tables).  All feature math runs on device
in fp16 (fp32 accumulation); host work is limited to sharding and index
preprocessing (edge sort/merge, degree counts, layout packing).

Key structure choices:
  - x is shipped feature-major ([256, nodes], fp16) so conv1 consumes it
    directly as the stationary operand -> no on-device transposes at all.
  - conv outputs are node-major (x^T/h^T blocks stationary, W moving), so
    aggregation lhsT chunks come straight from the conv PSUM.
  - conv2's root-broadcast rows contribute a rank-1 term: c2 = W2[128:]^T @
    relu(root) is computed once per graph/branch ([128,128]-cheap) and added
    during the PSUM evacuation, so conv2 contracts only 128 of 384 features.
  - all small weights ride in ONE packed DMA blob (HWDGE overhead is ~625ns
    per dma_start; 35 separate loads would burn ~22us of startup).
"""

import numpy as np

import concourse.bass as bass
import concourse.bacc as bacc
import concourse.mybir as mybir
import concourse.tile as tile
from concourse.bass_utils import run_bass_kernel_spmd

F32 = mybir.dt.float32
F16 = mybir.dt.float16
I16 = mybir.dt.int16
I32 = mybir.dt.int32

N, B, E = 65536, 128, 524288
IN, HID, OUT, NCLS = 256, 128, 128, 4
MAX_HOP = 10
NPG = N // B            # 512 nodes per graph
NCORES = 8
GPC = B // NCORES       # 16 graphs per core
NPCORE = N // NCORES    # 8192 nodes per core
NCHUNK = NPG // 128     # 4 x 128-node chunks per graph

AF = mybir.ActivationFunctionType
ALU = mybir.AluOpType


# --------------------------------------------------------------------------
# Packed constant-blob layout (shared by host packing and device slicing)
# --------------------------------------------------------------------------

def _f32_layout():
    """name -> (col0, rows, ncols); column-packed [128, CF32] f32 blob."""
    lay = {}
    c = 0

    def add(name, rows, ncols):
        nonlocal c
        lay[name] = (c, rows, ncols)
        c += ncols

    add("wu", 128, 128)
    add("ws", 128, 128)
    add("wd", 128, 128)
    add("wub", 128, 1)
    add("wsb", 128, 1)
    add("wdb", 128, 1)
    add("wu0b", 128, 1)
    add("ws0b", 128, 1)
    add("wd0b", 128, 1)
    add("b1_td", 128, 1)
    add("b1_bu", 128, 1)
    add("b2_td", 128, 1)
    add("b2_bu", 128, 1)
    add("wx", 128, 3 * (OUT + HID))      # [128, 3, 256] k-major
    add("wxb2", 128, 2)                  # [128, 2] (two 128-halves of wx_b)
    add("lu", 128, 1)
    add("ls", 128, 1)
    add("ld", 128, 1)
    add("fcw", 128, 4 * NCLS)            # [128, 4, 4] k-major
    add("id16x16", 16, 16)
    add("wu0", 1, 128)
    add("lub", 1, 1)
    add("lsb", 1, 1)
    add("ldb", 1, 1)
    add("fcb", 1, NCLS)
    add("rawab", 1, 2)
    add("nhop", 1, GPC)                  # int32 bits
    add("us", GPC, 600)
    return lay, c


def _f16_layout():
    lay = {}
    c = 0

    def add(name, ncols):
        nonlocal c
        lay[name] = (c, ncols)
        c += ncols

    add("w1_td", 2 * HID)                # [128, 2, 128] k-major
    add("w1_bu", 2 * HID)
    add("w2_td", 3 * OUT)                # [128, 3, 128] k-major
    add("w2_bu", 3 * OUT)
    return lay, c


F32LAY, CF32 = _f32_layout()
F16LAY, CF16 = _f16_layout()


# --------------------------------------------------------------------------
# Bass program (one core's SPMD program; all cores identical, shards differ)
# --------------------------------------------------------------------------

def build_program(W: int):
    nc = bacc.Bacc("TRN2", target_bir_lowering=False, debug=False,
                   num_devices=NCORES)

    xt_d = nc.dram_tensor("xt_sh", [IN, NPCORE], F16, kind="ExternalInput").ap()
    scat_d = nc.dram_tensor("scat", [GPC, 128, 2, 2, 2, W], I16,
                            kind="ExternalInput").ap()
    cb32_d = nc.dram_tensor("cb32", [128, CF32], F32, kind="ExternalInput").ap()
    cb16_d = nc.dram_tensor("cb16", [128, CF16], F16, kind="ExternalInput").ap()

    logp_d = nc.dram_tensor("out_logp", [GPC, NCLS], F32, kind="ExternalOutput").ap()
    usd_d = [nc.dram_tensor(f"out_{k}", [1, MAX_HOP * GPC], F32,
                            kind="ExternalOutput").ap() for k in ("uo", "so", "do")]

    from contextlib import ExitStack

    with tile.TileContext(nc) as tc, ExitStack() as ctx:
        const = ctx.enter_context(tc.tile_pool(name="const", bufs=1))
        xload = ctx.enter_context(tc.tile_pool(name="xload", bufs=3))
        sb = ctx.enter_context(tc.tile_pool(name="sb", bufs=2))
        sb3 = ctx.enter_context(tc.tile_pool(name="sb3", bufs=3))
        pers = ctx.enter_context(tc.tile_pool(name="pers", bufs=1))
        p_h = ctx.enter_context(tc.tile_pool(name="p_h", bufs=3, space="PSUM"))
        p_agg = ctx.enter_context(tc.tile_pool(name="p_agg", bufs=3, space="PSUM"))
        p_sm = ctx.enter_context(tc.tile_pool(name="p_sm", bufs=2, space="PSUM"))

        cb32 = const.tile([128, CF32], F32, tag="cb32")
        nc.sync.dma_start(cb32[:], cb32_d)
        cb16 = const.tile([128, CF16], F16, tag="cb16")
        nc.sync.dma_start(cb16[:], cb16_d)

        def c32(name, reshape=None):
            c0, rows, ncols = F32LAY[name]
            ap = cb32[0:rows, c0:c0 + ncols]
            if reshape:
                ap = ap.rearrange(reshape[0], **reshape[1])
            return ap

        def c16(name, k):
            c0, ncols = F16LAY[name]
            return cb16[:, c0:c0 + ncols].rearrange("p (k m) -> p k m", k=k)

        w1sb = [c16("w1_td", 2), c16("w1_bu", 2)]
        w2sb = [c16("w2_td", 3), c16("w2_bu", 3)]
        b1sb = [c32("b1_td"), c32("b1_bu")]
        b2sb = [c32("b2_td"), c32("b2_bu")]
        wxsb = c32("wx", ("p (k m) -> p k m", {"k": 3}))
        fcwsb = c32("fcw", ("p (k m) -> p k m", {"k": 4}))

        ones128 = const.tile([1, 128], F32, tag="ones128")
        nc.vector.memset(ones128[:], 1.0)

        # persistent accumulators
        fcrhs = [pers.tile([128, GPC], F32, tag=f"fcrhs{k}", name=f"fcrhs{k}")
                 for k in range(4)]
        UallT = pers.tile([128, MAX_HOP * GPC], F32, tag="UallT")
        SallT = pers.tile([128, MAX_HOP * GPC], F32, tag="SallT")
        DallT = pers.tile([128, MAX_HOP * GPC], F32, tag="DallT")

        # =================== hop chain (tiny, overlaps graph work) =========
        sig = sb.tile([1, 2], F32, tag="sig")
        nc.scalar.activation(sig[:], c32("rawab"), AF.Sigmoid)
        vals3 = sb.tile([1, 3], F32, tag="vals3")
        nc.vector.tensor_copy(vals3[:, 0:2], sig[:])
        t11 = sb.tile([1, 1], F32, tag="t11")
        nc.vector.tensor_tensor(t11[:], sig[:, 0:1], sig[:, 1:2], ALU.add)
        nc.vector.tensor_scalar(vals3[:, 2:3], t11[:], -1.0, 1.0, ALU.mult, ALU.add)
        ps3 = p_sm.tile([128, 3], F32, tag="psm")
        nc.tensor.matmul(ps3[:], ones128[:], vals3[:])
        scal_bc = pers.tile([128, 3], F32, tag="scal_bc")
        nc.vector.tensor_copy(scal_bc[:], ps3[:])

        us_sum = sb.tile([GPC, 1], F32, tag="us_sum")
        nc.vector.tensor_reduce(us_sum[:], c32("us"), mybir.AxisListType.X, ALU.add)
        ps_u0 = p_sm.tile([1, GPC], F32, tag="psm")
        nc.tensor.matmul(ps_u0[:], us_sum[:], c32("id16x16"))
        u0row = sb.tile([1, GPC], F32, tag="u0row")
        nc.scalar.copy(u0row[:], ps_u0[:])

        ps_U0 = p_sm.tile([128, GPC], F32, tag="psm")
        nc.tensor.matmul(ps_U0[:], c32("wu0"), u0row[:])
        U_prev = pers.tile([128, GPC], F32, tag="U_prev")
        nc.vector.tensor_scalar(U_prev[:], ps_U0[:], c32("wu0b"), None, ALU.add)
        S_prev = pers.tile([128, GPC], F32, tag="S_prev")
        nc.vector.tensor_copy(S_prev[:], c32("ws0b").broadcast_to([128, GPC]))
        D_prev = pers.tile([128, GPC], F32, tag="D_prev")
        nc.vector.tensor_copy(D_prev[:], c32("wd0b").broadcast_to([128, GPC]))

        for k in range(MAX_HOP):
            sl = slice(k * GPC, (k + 1) * GPC)
            tmpU = sb.tile([128, GPC], F32, tag="tmpU")
            nc.vector.tensor_scalar(tmpU[:], U_prev[:], scal_bc[:, 2:3], None, ALU.mult)
            psu = p_sm.tile([128, GPC], F32, tag="psm")
            nc.tensor.matmul(psu[:], c32("wu"), tmpU[:])
            nc.vector.tensor_scalar(UallT[:, sl], psu[:], c32("wub"), None, ALU.add)

            tmpS = sb.tile([128, GPC], F32, tag="tmpS")
            nc.vector.tensor_scalar(tmpS[:], UallT[:, sl], scal_bc[:, 0:1], None, ALU.mult)
            nc.vector.tensor_tensor(tmpS[:], tmpS[:], S_prev[:], ALU.add)
            pss = p_sm.tile([128, GPC], F32, tag="psm")
            nc.tensor.matmul(pss[:], c32("ws"), tmpS[:])
            nc.vector.tensor_scalar(SallT[:, sl], pss[:], c32("wsb"), None, ALU.add)

            tmpD = sb.tile([128, GPC], F32, tag="tmpD")
            nc.vector.tensor_scalar(tmpD[:], UallT[:, sl], scal_bc[:, 1:2], None, ALU.mult)
            nc.vector.tensor_tensor(tmpD[:], tmpD[:], D_prev[:], ALU.add)
            psd = p_sm.tile([128, GPC], F32, tag="psm")
            nc.tensor.matmul(psd[:], c32("wd"), tmpD[:])
            nc.vector.tensor_scalar(DallT[:, sl], psd[:], c32("wdb"), None, ALU.add)

            U_prev, S_prev, D_prev = UallT[:, sl], SallT[:, sl], DallT[:, sl]

        for allT, lw, lb, od in ((UallT, "lu", "lub", usd_d[0]),
                                 (SallT, "ls", "lsb", usd_d[1]),
                                 (DallT, "ld", "ldb", usd_d[2])):
            pso = p_sm.tile([1, MAX_HOP * GPC], F32, tag="psm")
            nc.tensor.matmul(pso[:], c32(lw), allT[:])
            osb = sb.tile([1, MAX_HOP * GPC], F32, tag="osb")
            nc.vector.tensor_scalar(osb[:], pso[:], c32(lb), None, ALU.add)
            nc.sync.dma_start(od, osb[:])

        # hop one-hot selection -> U_m/S_m/D_m  [128, GPC]
        nh1 = sb.tile([1, GPC], I32, tag="nh1")
        nc.vector.tensor_scalar(nh1[:], c32("nhop").bitcast(I32), 1, None, ALU.subtract)
        iota_t = sb.tile([1, MAX_HOP * GPC], I32, tag="iota")
        nc.gpsimd.iota(iota_t[:], [[1, MAX_HOP], [0, GPC]], channel_multiplier=0)
        oh_i = sb.tile([1, MAX_HOP * GPC], I32, tag="oh_i")
        nc.vector.tensor_tensor(
            oh_i[:].rearrange("p (h g) -> p h g", h=MAX_HOP),
            iota_t[:].rearrange("p (h g) -> p h g", h=MAX_HOP),
            nh1[:].unsqueeze(1).broadcast_to([1, MAX_HOP, GPC]),
            ALU.is_equal)
        oh_f = sb.tile([1, MAX_HOP * GPC], F32, tag="oh_f")
        nc.vector.tensor_copy(oh_f[:], oh_i[:])
        ps_oh = p_sm.tile([128, MAX_HOP * GPC], F32, tag="psm")
        nc.tensor.matmul(ps_oh[:], ones128[:], oh_f[:])
        ohB = sb.tile([128, MAX_HOP * GPC], F32, tag="ohB")
        nc.vector.tensor_copy(ohB[:], ps_oh[:])

        msel = []
        for allT, nm in ((UallT, "U"), (SallT, "S"), (DallT, "D")):
            msk = sb.tile([128, MAX_HOP * GPC], F32, tag="msk")
            nc.vector.tensor_tensor(msk[:], allT[:], ohB[:], ALU.mult)
            mt = pers.tile([128, GPC], F32, tag=f"m_{nm}")
            nc.vector.tensor_reduce(mt[:], msk[:].rearrange("p (h g) -> p g h", h=MAX_HOP),
                                    mybir.AxisListType.X, ALU.add)
            msel.append(mt)

        xgT = []
        for m in range(2):
            psx = p_sm.tile([128, GPC], F32, tag="psm")
            for k in range(3):
                nc.tensor.matmul(psx[:], wxsb[:, k, 128 * m:128 * (m + 1)], msel[k][:],
                                 start=(k == 0), stop=(k == 2))
            xg_m = pers.tile([128, GPC], F32, tag=f"xg{m}", name=f"xg{m}")
            nc.vector.tensor_scalar(xg_m[:], psx[:], cb32[0:128, F32LAY["wxb2"][0] + m:
                                                          F32LAY["wxb2"][0] + m + 1],
                                    None, ALU.add)
            xgT.append(xg_m)

        # =================== per-graph GNN ================================
        for g in range(GPC):
            # feature-major x chunks, fp16, straight from HBM
            xT = []
            for i in range(2):
                xt_i = xload.tile([128, NPG], F16, tag="xT", name=f"xT{i}")
                nc.sync.dma_start(
                    xt_i[:], xt_d[128 * i:128 * (i + 1), g * NPG:(g + 1) * NPG])
                xT.append(xt_i)
            scat_sb = xload.tile([128, 2, 2, 2, W], I16, tag="scat_sb")
            nc.sync.dma_start(scat_sb[:], scat_d[g])

            # root feature column (node 0), relu'd, + [128,128] broadcasts
            rbc = []
            for i in range(2):
                rroot = sb.tile([128, 1], F16, tag="rroot")
                nc.scalar.activation(rroot[:], xT[i][:, 0:1], AF.Relu)
                rb = sb.tile([128, 128], F16, tag="rbc")
                nc.vector.tensor_copy(rb[:], rroot[:].broadcast_to([128, 128]))
                rbc.append(rb)

            for b in range(2):
                # adjacency build (fp16): 2 merged scatters of [128, 1024]
                # (chunk-pair layout: [128, 2, 1024] == [128, 4, 512])
                ahat = sb3.tile([128, 2, 2 * NPG], F16, tag="ahat")
                for m in range(2):
                    nc.gpsimd.local_scatter(
                        ahat[:, m, :],
                        scat_sb[:, b, m, 1, :].bitcast(F16),
                        scat_sb[:, b, m, 0, :],
                        channels=128, num_elems=2 * NPG, num_idxs=W)
                ahat = ahat.rearrange("p m (s n) -> p (m s) n", n=NPG)

                # conv1, node-major: x^T blocks stationary, W1 moving
                ph1 = p_h.tile([128, NCHUNK, HID], F32, tag="ph")
                for j in range(NCHUNK):
                    for k in range(2):
                        nc.tensor.matmul(ph1[:, j, :],
                                         xT[k][:, 128 * j:128 * (j + 1)],
                                         w1sb[b][:, k, :],
                                         start=(k == 0), stop=(k == 1))
                h1n = sb.tile([128, NCHUNK, HID], F16, tag="h1n")
                if b == 0:
                    nc.scalar.copy(h1n[:], ph1[:])
                else:
                    nc.vector.tensor_copy(h1n[:], ph1[:])

                # aggregate 1
                pag1 = p_agg.tile([128, NPG], F32, tag="pagg")
                for c in range(NCHUNK):
                    nc.tensor.matmul(pag1[:], h1n[:, c, :], ahat[:, c, :],
                                     start=(c == 0), stop=(c == NCHUNK - 1))

                # conv2 inputs: relu(agg1 + b1) (feat rows 0-127) + rank-1 root term
                rhs2_0 = sb.tile([128, NPG], F16, tag="rhs2_0")
                if b == 0:
                    nc.vector.tensor_scalar(rhs2_0[:], pag1[:], b1sb[b], 0.0,
                                            ALU.add, ALU.max)
                else:
                    nc.scalar.activation(rhs2_0[:], pag1[:], AF.Relu, bias=b1sb[b])
                x2root = sb.tile([128, 1], F32, tag="x2root")
                nc.vector.tensor_scalar(x2root[:], pag1[:, 0:1], b1sb[b], None, ALU.add)

                # c2 = W2[128:384]^T @ relu(root): all rows equal c2
                pc2 = p_sm.tile([128, HID], F32, tag="psm")
                for k in (1, 2):
                    nc.tensor.matmul(pc2[:], rbc[k - 1][:], w2sb[b][:, k, :],
                                     start=(k == 1), stop=(k == 2))
                c2sb = sb.tile([128, HID], F32, tag="c2sb")
                nc.vector.tensor_copy(c2sb[:], pc2[:])

                # conv2, node-major, single 128-feature contraction
                ph2 = p_h.tile([128, NCHUNK, OUT], F32, tag="ph")
                for j in range(NCHUNK):
                    nc.tensor.matmul(ph2[:, j, :],
                                     rhs2_0[:, 128 * j:128 * (j + 1)],
                                     w2sb[b][:, 0, :])
                h2n = sb.tile([128, NCHUNK, OUT], F16, tag="h2n")
                nc.vector.tensor_tensor(h2n[:], ph2[:],
                                        c2sb[:].unsqueeze(1).broadcast_to(
                                            [128, NCHUNK, OUT]),
                                        ALU.add)

                # aggregate 2
                pag2 = p_agg.tile([128, NPG], F32, tag="pagg")
                for c in range(NCHUNK):
                    nc.tensor.matmul(pag2[:], h2n[:, c, :], ahat[:, c, :],
                                     start=(c == 0), stop=(c == NCHUNK - 1))

                # out = relu(agg2 + b2); pooled mean via accum_out
                out2 = sb.tile([128, NPG], F16, tag="out2")
                pool_sum = sb.tile([128, 1], F32, tag="pool_sum")
                nc.scalar.activation(out2[:], pag2[:], AF.Relu, bias=b2sb[b],
                                     accum_out=pool_sum[:])

                base = 2 if b == 0 else 0
                nc.vector.tensor_scalar(fcrhs[base][:, g:g + 1], pool_sum[:],
                                        1.0 / NPG, None, ALU.mult)
                nc.vector.tensor_copy(fcrhs[base + 1][:, g:g + 1], x2root[:])

        # =================== final FC + log_softmax =======================
        for k in range(4):
            nc.vector.tensor_tensor(fcrhs[k][:], fcrhs[k][:], xgT[k % 2][:], ALU.add)

        ps_log = p_sm.tile([GPC, NCLS], F32, tag="psm")
        for k in range(4):
            nc.tensor.matmul(ps_log[:], fcrhs[k][:], fcwsb[:, k, :],
                             start=(k == 0), stop=False)
        nc.tensor.matmul(ps_log[:], ones128[:, 0:GPC], c32("fcb"),
                         start=False, stop=True)

        negmax = sb.tile([GPC, 1], F32, tag="negmax")
        nc.vector.tensor_reduce(negmax[:], ps_log[:], mybir.AxisListType.X,
                                ALU.max, negate=True)
        exp_sb = sb.tile([GPC, NCLS], F32, tag="exp_sb")
        nc.scalar.activation(exp_sb[:], ps_log[:], AF.Exp, bias=negmax[:])
        ssum = sb.tile([GPC, 1], F32, tag="ssum")
        nc.vector.tensor_reduce(ssum[:], exp_sb[:], mybir.AxisListType.X, ALU.add)
        lse = sb.tile([GPC, 1], F32, tag="lse")
        nc.scalar.activation(lse[:], ssum[:], AF.Ln)
        mpl = sb.tile([GPC, 1], F32, tag="mpl")
        nc.vector.tensor_tensor(mpl[:], lse[:], negmax[:], ALU.subtract)
        lp_sb = sb.tile([GPC, NCLS], F32, tag="lp_sb")
        nc.vector.tensor_scalar(lp_sb[:], ps_log[:], mpl[:], None, ALU.subtract)
        nc.sync.dma_start(logp_d, lp_sb[:])

    nc.compile()
    return nc


# --------------------------------------------------------------------------
# Host-side preprocessing: shard + scatter-table build + blob packing
# --------------------------------------------------------------------------

def _scatter_tables(src, dst):
    """Padded per-merged-row (idx, val) tables for both branches.

    Each scatter call covers a 256-node chunk-PAIR: merged row
    R = (node//256)*128 + node%128 holds the node's receiver columns,
    shifted by +512 for the odd 128-chunk of the pair (num_elems=1024)."""
    loop = np.arange(N, dtype=np.int64)
    s2 = np.concatenate([src, loop])
    d2 = np.concatenate([dst, loop])
    NR = N // 2

    out = []
    W = 2
    for bs, bd in ((s2, d2), (d2, s2)):
        # row = message provider (bs), col = receiver local idx (bd % NPG)
        deg = np.bincount(bd, minlength=N).astype(np.float64)
        dinv = 1.0 / np.sqrt(deg)  # deg >= 1 thanks to self loops
        key = bs * NPG + (bd % NPG)
        uk, cnt = np.unique(key, return_counts=True)
        rows = uk // NPG
        cols = (uk % NPG).astype(np.int64)
        cols_g = (rows // NPG) * NPG + cols
        vals = cnt * dinv[rows] * dinv[cols_g]
        mrow = (rows // 256) * 128 + rows % 128
        mcol = cols + 512 * ((rows % 256) // 128)
        rc = np.bincount(mrow, minlength=NR)
        W = max(W, int(rc.max()))
        out.append((mrow, mcol, vals, rc))

    W = (W + 1) // 2 * 2
    idx_a = np.full((2, NR, W), -1, np.int16)
    val_a = np.zeros((2, NR, W), np.float16)
    for i, (mrow, mcol, vals, rc) in enumerate(out):
        offs = np.zeros(NR + 1, np.int64)
        np.cumsum(rc, out=offs[1:])
        # uk sort order is by (row, col-within-node); entries of the two
        # sub-chunks interleave by original node, but bincount offsets are
        # per merged row, so recompute positions per merged row:
        order = np.argsort(mrow, kind="stable")
        mrow = mrow[order]; mcol = mcol[order]; vals = vals[order]
        pos = np.arange(len(mrow)) - offs[mrow]
        idx_a[i, mrow, pos] = mcol.astype(np.int16)
        val_a[i, mrow, pos] = vals.astype(np.float16)
    return idx_a, val_a, W


def _pack_blobs(params, user_state, num_hop):
    p = {k: np.asarray(v, np.float32) for k, v in params.items()}
    b32 = np.zeros((128, CF32), np.float32)

    def put(name, arr):
        c0, rows, ncols = F32LAY[name]
        arr = np.asarray(arr, np.float32).reshape(rows, ncols)
        b32[0:rows, c0:c0 + ncols] = arr

    put("wu", p["wu_w"]); put("ws", p["ws_w"]); put("wd", p["wd_w"])
    put("wub", p["wu_b"].reshape(128, 1)); put("wsb", p["ws_b"].reshape(128, 1))
    put("wdb", p["wd_b"].reshape(128, 1))
    put("wu0b", p["wu0_b"].reshape(128, 1)); put("ws0b", p["ws0_b"].reshape(128, 1))
    put("wd0b", p["wd0_b"].reshape(128, 1))
    put("b1_td", p["td_b1"].reshape(128, 1)); put("b1_bu", p["bu_b1"].reshape(128, 1))
    put("b2_td", p["td_b2"].reshape(128, 1)); put("b2_bu", p["bu_b2"].reshape(128, 1))
    # wx [384, 256] -> [128, 3, 256] k-major
    put("wx", p["wx_w"].reshape(3, 128, OUT + HID).transpose(1, 0, 2).reshape(128, -1))
    put("wxb2", p["wx_b"].reshape(2, 128).T)
    put("lu", p["lu_w"]); put("ls", p["ls_w"]); put("ld", p["ld_w"])
    put("fcw", p["fc_w"].reshape(4, 128, NCLS).transpose(1, 0, 2).reshape(128, -1))
    put("id16x16", np.eye(GPC, dtype=np.float32))
    put("wu0", p["wu0_w"].reshape(1, 128))
    put("lub", p["lu_b"].reshape(1, 1)); put("lsb", p["ls_b"].reshape(1, 1))
    put("ldb", p["ld_b"].reshape(1, 1))
    put("fcb", p["fc_b"].reshape(1, NCLS))
    put("rawab", np.array([[p["raw_alpha"][0], p["raw_beta"][0]]], np.float32))

    b16 = np.zeros((128, CF16), np.float16)

    def put16(name, arr, k):
        c0, ncols = F16LAY[name]
        a = np.asarray(arr, np.float32).reshape(k, 128, -1).transpose(1, 0, 2)
        b16[:, c0:c0 + ncols] = a.reshape(128, ncols).astype(np.float16)

    put16("w1_td", p["td_w1"], 2); put16("w1_bu", p["bu_w1"], 2)
    put16("w2_td", p["td_w2"], 3); put16("w2_bu", p["bu_w2"], 3)

    # per-core f32 blobs differ in nhop/us
    blobs32 = []
    c0, _, _ = F32LAY["nhop"]
    cu, _, _ = F32LAY["us"]
    for c in range(NCORES):
        bb = b32.copy()
        nh = np.asarray(num_hop[c * GPC:(c + 1) * GPC], np.int32)
        bb[0, c0:c0 + GPC] = nh.view(np.float32)
        bb[0:GPC, cu:cu + 600] = np.asarray(
            user_state[c * GPC:(c + 1) * GPC], np.float32).reshape(GPC, 600)
        blobs32.append(bb)
    return blobs32, b16


def _prep_inputs(x, user_state, params, edge_index, batch, num_hop):
    x = np.asarray(x, np.float32)
    user_state = np.asarray(user_state, np.float32)
    src = np.asarray(edge_index[0], np.int64)
    dst = np.asarray(edge_index[1], np.int64)
    num_hop = np.asarray(num_hop, np.int64)
    assert (src // NPG == dst // NPG).all(), "edges must be intra-graph"

    idx_a, val_a, W = _scatter_tables(src, dst)
    idx_r = idx_a.reshape(2, NCORES, GPC, 2, 128, W)
    val_r = val_a.reshape(2, NCORES, GPC, 2, 128, W).view(np.int16)
    scat = np.empty((NCORES, GPC, 128, 2, 2, 2, W), np.int16)
    for b in range(2):
        # [core, g, m, p, w] -> [core, g, p, m, w]
        scat[:, :, :, b, :, 0, :] = idx_r[b].transpose(0, 1, 3, 2, 4)
        scat[:, :, :, b, :, 1, :] = val_r[b].transpose(0, 1, 3, 2, 4)

    xt16 = np.ascontiguousarray(x.T.astype(np.float16))  # [256, N]
    blobs32, b16 = _pack_blobs(params, user_state, num_hop)

    in_maps = []
    for c in range(NCORES):
        in_maps.append({
            "xt_sh": np.ascontiguousarray(xt16[:, c * NPCORE:(c + 1) * NPCORE]),
            "scat": np.ascontiguousarray(scat[c]),
            "cb32": blobs32[c],
            "cb16": b16,
        })
    return in_maps, W


def _assemble(results):
    logp = np.concatenate([r["out_logp"] for r in results], 0)
    outs = []
    for key in ("out_uo", "out_so", "out_do"):
        per = [r[key].reshape(MAX_HOP, GPC).T.reshape(GPC, MAX_HOP, 1)
               for r in results]
        outs.append(np.concatenate(per, 0))
    return (logp, outs[0], outs[1], outs[2])


_CACHE = {}


def kernel(x, user_state, params, edge_index, batch, num_hop, _want_stats=False):
    in_maps, W = _prep_inputs(x, user_state, params, edge_index, batch, num_hop)
    if W not in _CACHE:
        _CACHE[W] = build_program(W)
    nc = _CACHE[W]
    res = run_bass_kernel_spmd(nc, in_maps, core_ids=list(range(NCORES)))
    out = _assemble(res.results)
    if _want_stats:
        return out, res
    return out
